# revision 4
# baseline (speedup 1.0000x reference)
"""StyleGAN2 fused upsample2x + 3x3 conv + FIR(1,3,3,1) + bias — TRN2 Bass kernel.

Decomposition (validated in proto.py):
  full op = conv_transpose(x, w, stride 2) then FIR. The FIR factors as
  (1,3,3,1) = (1,1)*(1,1)*(1,1) per axis (box-filter cubed). One horizontal
  box is folded into the conv weights (wh = w *_cols (1,1), 4 wide), giving a
  12-tap-plane parity conv on the coarse grid (PE, fp16, 1 cyc/row). The
  remaining 3 vertical + 2 horizontal box passes run as 2-tap tensor_adds on
  DVE (fp16 2x mode, parity-split planes keep every access 4B-aligned) with
  the two unavoidably odd-aligned ops per pass pair on GPSIMD. Bias enters as
  b/8 on the odd-odd lattice (one extra 1-row matmul + pad-row fill).
  Output is stored as 4 parity planes per image; host interleaves.

Per-core work: 2 images x 2 output-channel halves. 8 cores data-parallel.
"""

import sys

sys.path.insert(0, "/opt/trn_rl_repo")

import numpy as np

import concourse.bacc as bacc
import concourse.mybir as mybir
import concourse.tile as tile
from concourse.bass_utils import run_bass_kernel_spmd

N_CORES = 8
IMGS = 16
IMG_PER_CORE = IMGS // N_CORES  # 2
C = 256
O = 256
H = W = 64
M1 = H + 1  # 65
HP = H + 2  # 66
NK = 2
NM = 2
BLK = 7  # conv block rows (one PSUM bank: 7*65=455 fp32 <= 512)

# Tap tables in storage coords (see proto.py). (di, vo): weight-row di of the
# 3-row kernel, xp row offset vo. (djp, ho): col djp of the 4-wide folded
# kernel wh, xp col offset ho.
VT = {0: [(0, 0), (2, 1)], 1: [(1, 1)]}
HT = {0: [(0, 0), (2, 1)], 1: [(1, 0), (3, 1)]}
PLANES = [(0, 0), (0, 1), (1, 0), (1, 1)]
TAPS = {(rp, cp): [(di, vo, djp, ho) for (di, vo) in VT[rp] for (djp, ho) in HT[cp]]
        for (rp, cp) in PLANES}
TAP_BASE = {}
_acc = 0
for _p in PLANES:
    TAP_BASE[_p] = _acc
    _acc += len(TAPS[_p])
N_TAP = _acc  # 12


def _chunk(m, plane, t_i, k):
    return (m * N_TAP + TAP_BASE[plane] + t_i) * NK + k


_compiled = None
LAST_RESULTS = None


def _build():
    nc = bacc.Bacc(None, target_bir_lowering=False, debug=False)
    dt = mybir.dt
    f16 = dt.float16
    ADD = mybir.AluOpType.add

    xp_d = nc.dram_tensor("xp", (IMG_PER_CORE, NK, 128, HP * HP), f16,
                          kind="ExternalInput")
    wt_d = nc.dram_tensor("wt", (128, NM * N_TAP * NK * 128), f16,
                          kind="ExternalInput")
    bl_d = nc.dram_tensor("bl", (1, O), f16, kind="ExternalInput")
    bcol_d = nc.dram_tensor("bcol", (128, NM), dt.float32, kind="ExternalInput")
    out_d = nc.dram_tensor("out", (IMG_PER_CORE, O, 2, H, 2, W), f16,
                           kind="ExternalOutput")

    with tile.TileContext(nc) as tc:
        with (
            tc.tile_pool(name="wpool", bufs=1) as wpool,
            tc.tile_pool(name="xpool", bufs=1) as xpool,
            tc.tile_pool(name="zpool", bufs=2) as zpool,
            tc.tile_pool(name="apool", bufs=1) as apool,
            tc.tile_pool(name="bpool", bufs=1) as bpool,
            tc.tile_pool(name="psum", bufs=8, space="PSUM") as psum_pool,
        ):
            wt_t = wpool.tile([128, NM * N_TAP * NK * 128], f16, tag="wt")
            nc.sync.dma_start(wt_t[:], wt_d.ap()[:])
            bl_t = wpool.tile([1, O], f16, tag="bl")
            nc.sync.dma_start(bl_t[:], bl_d.ap()[:])
            bcol_t = wpool.tile([128, NM], dt.float32, tag="bcol")
            nc.sync.dma_start(bcol_t[:], bcol_d.ap()[:])
            ones_t = wpool.tile([1, BLK * M1], f16, tag="ones")
            nc.vector.memset(ones_t[:], 1.0)
            onesp_t = wpool.tile([128, 1, HP], f16, tag="onesp")
            nc.vector.memset(onesp_t[:], 1.0)

            xp_t = {}
            for img in range(IMG_PER_CORE):
                for k in range(NK):
                    t = xpool.tile([128, HP, HP], f16, tag=f"xp{img}{k}")
                    nc.sync.dma_start(
                        t[:], xp_d.ap()[img, k].rearrange("p (h w) -> p h w", h=HP))
                    xp_t[img, k] = t

            for img in range(IMG_PER_CORE):
                for m in range(NM):
                    unit = img * NM + m
                    # z-parity planes: plane p=rp*2+cp at zt[:, p]; data rows
                    # at 1..nrows, so rp=1 pad rows 0 and 65 stay untouched.
                    zt = zpool.tile([128, 4, HP, HP], f16, tag="zt")
                    if unit < 2:
                        # one-time pad init per physical buffer (buf = m here)
                        nc.vector.memset(zt[:, 2, 0:1, :], 0.0)
                        nc.vector.memset(zt[:, 2, M1:M1 + 1, :], 0.0)
                        nc.vector.tensor_scalar_mul(
                            zt[:, 3, 0:1, :], onesp_t[:], bcol_t[:, m:m + 1])
                        nc.vector.tensor_scalar_mul(
                            zt[:, 3, M1:M1 + 1, :], onesp_t[:], bcol_t[:, m:m + 1])

                    # ---- conv: 12 tap-planes + bias, blocks of 7 rows ----
                    nblk = (M1 + BLK - 1) // BLK
                    for b in range(nblk):
                        r0 = BLK * b
                        for p_i, (rp, cp) in enumerate(PLANES):
                            nrows = M1 if rp == 0 else H
                            nr = min(BLK, nrows - r0)
                            if nr <= 0:
                                continue
                            ps = psum_pool.tile([128, BLK, M1], dt.float32)
                            tp = TAPS[(rp, cp)]
                            has_bias = (rp, cp) == (1, 1)
                            nmm = len(tp) * NK + (1 if has_bias else 0)
                            i_mm = 0
                            for k in range(NK):
                                for t_i, (di, vo, djp, ho) in enumerate(tp):
                                    idx = _chunk(m, (rp, cp), t_i, k)
                                    rhs = xp_t[img, k][
                                        :, r0 + vo:r0 + vo + nr, ho:ho + M1]
                                    nc.tensor.matmul(
                                        ps[:, 0:nr, :],
                                        wt_t[:, idx * 128:(idx + 1) * 128],
                                        rhs,
                                        start=(i_mm == 0),
                                        stop=(i_mm == nmm - 1),
                                    )
                                    i_mm += 1
                            if has_bias:
                                nc.tensor.matmul(
                                    ps[:, 0:nr, :],
                                    bl_t[0:1, m * 128:(m + 1) * 128],
                                    ones_t[0:1, 0:nr * M1],
                                    start=False, stop=True,
                                )
                            nc.scalar.activation(
                                zt[:, p_i, 1 + r0:1 + r0 + nr, 0:M1],
                                ps[:, 0:nr, :],
                                mybir.ActivationFunctionType.Copy,
                            )

                    # ---- FIR cascade: V1 V2 V3 (rows), H2 H3 (cols) ------
                    # A pool tags: te0 te1 to0 to1 (t, reused by v, then c)
                    # B pool tags: ue0 ue1 uo0 uo1 (u, reused by s)
                    te, to, ue, uo = {}, {}, {}, {}
                    for cp in range(2):
                        te[cp] = apool.tile([128, M1, HP], f16, tag=f"te{cp}", name=f"te{cp}")
                        to[cp] = apool.tile([128, M1, HP], f16, tag=f"to{cp}", name=f"to{cp}")
                    for cp in range(2):
                        nc.vector.tensor_add(
                            te[cp][:, 0:M1, 0:M1],
                            zt[:, cp, 1:HP, 0:M1], zt[:, 2 + cp, 1:HP, 0:M1])
                        nc.vector.tensor_add(
                            to[cp][:, 0:M1, 0:M1],
                            zt[:, 2 + cp, 0:M1, 0:M1], zt[:, cp, 1:HP, 0:M1])
                    for cp in range(2):
                        ue[cp] = bpool.tile([128, M1, HP], f16, tag=f"ue{cp}", name=f"ue{cp}")
                        uo[cp] = bpool.tile([128, M1, HP], f16, tag=f"uo{cp}", name=f"uo{cp}")
                        nc.vector.tensor_add(
                            ue[cp][:, 0:H, 0:M1],
                            te[cp][:, 0:H, 0:M1], to[cp][:, 1:M1, 0:M1])
                        nc.vector.tensor_add(
                            uo[cp][:, 0:M1, 0:M1],
                            to[cp][:, 0:M1, 0:M1], te[cp][:, 0:M1, 0:M1])
                    ve, vo_ = {}, {}
                    for cp in range(2):
                        ve[cp] = apool.tile([128, M1, HP], f16, tag=f"te{cp}", name=f"ve{cp}")
                        vo_[cp] = apool.tile([128, M1, HP], f16, tag=f"to{cp}", name=f"vo{cp}")
                        nc.vector.tensor_add(
                            ve[cp][:, 0:H, 0:M1],
                            ue[cp][:, 0:H, 0:M1], uo[cp][:, 1:M1, 0:M1])
                        nc.vector.tensor_add(
                            vo_[cp][:, 0:H, 0:M1],
                            uo[cp][:, 0:H, 0:M1], ue[cp][:, 0:H, 0:M1])

                    for rho, vv in (("e", ve), ("o", vo_)):
                        r_i = 0 if rho == "e" else 1
                        se = bpool.tile([128, M1, HP], f16, tag=f"ue{r_i}")
                        so = bpool.tile([128, M1, HP], f16, tag=f"uo{r_i}")
                        # s_e: odd-aligned second read -> GPSIMD
                        nc.gpsimd.tensor_add(
                            se[:, 0:H, 0:H],
                            vv[0][:, 0:H, 0:H], vv[1][:, 0:H, 1:M1])
                        nc.vector.tensor_add(
                            so[:, 0:H, 0:M1],
                            vv[1][:, 0:H, 0:M1], vv[0][:, 0:H, 0:M1])
                        ce = apool.tile([128, M1, HP], f16, tag=f"te{r_i}")
                        co = apool.tile([128, M1, HP], f16, tag=f"to{r_i}")
                        nc.gpsimd.tensor_add(
                            ce[:, 0:H, 0:H],
                            se[:, 0:H, 0:H], so[:, 0:H, 1:M1])
                        nc.vector.tensor_add(
                            co[:, 0:H, 0:H],
                            so[:, 0:H, 0:H], se[:, 0:H, 0:H])
                        rp_idx = 1 if rho == "e" else 0
                        osl = out_d.ap()[img, m * 128:(m + 1) * 128, rp_idx]
                        nc.sync.dma_start(osl[:, :, 1, :], ce[:, 0:H, 0:H])
                        nc.sync.dma_start(osl[:, :, 0, :], co[:, 0:H, 0:H])

    nc.compile()
    return nc


def _host_prep(x, w, b):
    w64 = w.astype(np.float64)
    wh = np.zeros((O, C, 3, 4))
    wh[:, :, :, :3] += w64
    wh[:, :, :, 1:] += w64
    wh /= 16.0

    wt = np.empty((128, NM * N_TAP * NK, 128), np.float16)
    for m in range(NM):
        for plane in PLANES:
            for t_i, (di, vo, djp, ho) in enumerate(TAPS[plane]):
                for k in range(NK):
                    idx = _chunk(m, plane, t_i, k)
                    wt[:, idx, :] = wh[m * 128:(m + 1) * 128,
                                       k * 128:(k + 1) * 128, di, djp].T
    wt = np.ascontiguousarray(wt.reshape(128, -1))

    b8 = (b.astype(np.float64) / 8.0).astype(np.float16)
    bl = np.ascontiguousarray(b8.reshape(1, O))
    bcol = np.ascontiguousarray(b8.reshape(NM, 128).T.astype(np.float32))

    xp = np.pad(x, ((0, 0), (0, 0), (1, 1), (1, 1))).astype(np.float16)
    xp = np.ascontiguousarray(
        xp.reshape(N_CORES, IMG_PER_CORE, NK, 128, HP * HP))
    return xp, wt, bl, bcol


def kernel(x, w, b):
    global _compiled, LAST_RESULTS
    if _compiled is None:
        _compiled = _build()
    nc = _compiled

    x = np.asarray(x, dtype=np.float32)
    w = np.asarray(w, dtype=np.float32)
    b = np.asarray(b, dtype=np.float32)

    xp, wt, bl, bcol = _host_prep(x, w, b)
    in_maps = [
        {"xp": xp[core], "wt": wt, "bl": bl, "bcol": bcol}
        for core in range(N_CORES)
    ]
    try:
        res = run_bass_kernel_spmd(nc, in_maps, list(range(N_CORES)))
    except ModuleNotFoundError:
        import os

        os.environ["BASS_NEVER_TRACE"] = "1"
        res = run_bass_kernel_spmd(nc, in_maps, list(range(N_CORES)))
    LAST_RESULTS = res
    blk = np.concatenate(
        [res.results[i]["out"] for i in range(N_CORES)], axis=0)
    # blk: [16, 256, rp, 64, cp, 64] -> [16, 256, 128, 128]
    full = blk.transpose(0, 1, 3, 2, 5, 4).reshape(IMGS, O, 2 * H, 2 * W)
    return np.ascontiguousarray(full.astype(np.float32))
